# revision 8
# baseline (speedup 1.0000x reference)
"""Trainium2 Bass kernel for nn_ContentSelector (topk_masking).

Math refactoring (validated against the reference in proto.py):
  - The attention query term h @ W[:dq] adds a per-batch constant to every
    score, and softmax is shift-invariant => attention weights are
    independent of h. ent_ctx / sent_ctx are therefore step-invariant and
    computed once. Biases bae/bas shift scores uniformly (dropped); bp and
    the sigmoid are monotonic (argmax-invariant, dropped).
  - sent_ctx is only consumed through q_s = sent_ctx . wp_s
      q_s = sum_n softmax(s)_n * t_n,  s_n = sents[n].was_m, t_n = sents[n].wp_s
    so sents needs only two matvecs (no [B,D] weighted sum).
  - top_idx = argmax_b p_b selects one of the first 8 sentences; output rows
    are exact f32 copies of input rows.

Distribution: data-parallel over batch for the attention phase (core c owns
batch c); the LSTM weight matrix is output-dim sharded (core c owns 128 of
each gate's 1024 dims) with one small AllGather of (h-slice^T, partial
h.wp_h) per recurrence step. Scores z = h.wp_h + q are computed redundantly
on every core so the argmax needs no extra communication.

On-chip dataflow: f32 HBM reads are cast to bf16 during DMA; score matvecs
run as DVE tensor_tensor multiplies + ACT copy-with-accum row sums (the
fused tensor_tensor_reduce custom-DVE op crashes this runtime); weighted
sums / gates / transposes run on the tensor engine with f32 PSUM accum.
"""
import numpy as np

B = 8
NS = 4096
NE = 1024
D = 1024
N_CORES = 8
KCH = 25          # 24 contraction chunks of 128 + 1 bias chunk
KPAD = KCH * 128  # 3200

_CACHE = {}


def _build():
    import concourse.bacc as bacc
    import concourse.bass as bass
    import concourse.mybir as mybir
    import concourse.tile as tile
    from concourse.bass_isa import ReduceOp

    dt = mybir.dt
    AF = mybir.ActivationFunctionType
    OP = mybir.AluOpType

    nc = bacc.Bacc(
        "TRN2", target_bir_lowering=False, debug=False,
        enable_asserts=True, num_devices=N_CORES,
    )

    sents = nc.dram_tensor("sents", [NS, D], dt.float32, kind="ExternalInput").ap()
    ents = nc.dram_tensor("ents", [NE, D], dt.float32, kind="ExternalInput").ap()
    s8 = nc.dram_tensor("s8", [64, D], dt.float32, kind="ExternalInput").ap()
    wsl = nc.dram_tensor("wsl", [512, KPAD], dt.float32, kind="ExternalInput").ap()
    wvecs = nc.dram_tensor("wvecs", [1, 8, D], dt.float32, kind="ExternalInput").ap()
    out = nc.dram_tensor("out", [B, 3, D], dt.float32, kind="ExternalOutput").ap()

    NSC = NS // 128  # 32 sent chunks
    NEC = NE // 128  # 8 entity chunks

    with tile.TileContext(nc) as tc:
        with (
            tc.tile_pool(name="sb", bufs=1) as sb,
            tc.tile_pool(name="ring", bufs=4) as ring,
            tc.tile_pool(name="psA", bufs=2, space="PSUM") as psA,
            tc.tile_pool(name="psB", bufs=2, space="PSUM") as psB,
            tc.tile_pool(name="psC", bufs=2, space="PSUM") as psC,
            tc.tile_pool(name="dram", bufs=1, space="DRAM") as dram,
        ):
            # ---------------- constants ----------------
            ident_f = sb.tile([128, 128], dt.float32)
            onesq = sb.tile([128, 128], dt.float32)
            nc.vector.memset(onesq, 1.0)
            nc.gpsimd.affine_select(
                out=ident_f, in_=onesq, pattern=[[-1, 128]],
                compare_op=OP.is_equal, fill=0.0, base=0, channel_multiplier=1,
            )
            ident_b = sb.tile([128, 128], dt.bfloat16)
            nc.scalar.copy(out=ident_b, in_=ident_f)

            ones128 = sb.tile([128, 1], dt.float32)
            nc.vector.memset(ones128, 1.0)

            # bmask[p, m] = 1 iff 0 <= p - 8m < 8   (only is_ge is implemented)
            bmask = sb.tile([64, 8], dt.float32)
            bm_a = sb.tile([64, 8], dt.float32)
            bm_b = sb.tile([64, 8], dt.float32)
            nc.gpsimd.affine_select(
                out=bm_a, in_=onesq[0:64, 0:8], pattern=[[-8, 8]],
                compare_op=OP.is_ge, fill=0.0, base=0, channel_multiplier=1,
            )
            nc.gpsimd.affine_select(
                out=bm_b, in_=onesq[0:64, 0:8], pattern=[[8, 8]],
                compare_op=OP.is_ge, fill=0.0, base=7, channel_multiplier=-1,
            )
            nc.vector.tensor_tensor(out=bmask, in0=bm_a, in1=bm_b, op=OP.mult)

            bias_chunk = sb.tile([128, 8], dt.bfloat16)
            nc.vector.memset(bias_chunk, 0.0)
            nc.vector.memset(bias_chunk[0:1, :], 1.0)

            # small weight vectors, replicated across partitions in bf16
            wv = sb.tile([1, 8, D], dt.float32)
            nc.sync.dma_start(out=wv, in_=wvecs)

            def rep_bf(row):
                r = sb.tile([1, D], dt.bfloat16, tag=f"repb{row}")
                nc.scalar.copy(out=r, in_=wv[:, row, :])
                full = sb.tile([128, D], dt.bfloat16, tag=f"repf{row}")
                nc.gpsimd.partition_broadcast(out_ap=full, in_ap=r, channels=128)
                return full

            wrep_s = rep_bf(0)   # was_m
            wrep_t = rep_bf(1)   # wp_s
            wrep_e = rep_bf(2)   # wae_m
            # wp_h slice for this core, on 8 partitions (f32)
            wph_row = sb.tile([1, 128], dt.float32)
            nc.vector.tensor_copy(wph_row, wv[:, 4, 0:128])
            wph8 = sb.tile([8, 128], dt.float32)
            nc.gpsimd.partition_broadcast(out_ap=wph8, in_ap=wph_row, channels=8)

            # ---------------- candidate sentences ----------------
            s8f = sb.tile([64, D], dt.float32)
            nc.sync.dma_start(out=s8f, in_=s8)
            s8b = sb.tile([64, D], dt.bfloat16)
            nc.scalar.copy(out=s8b, in_=s8f)
            selcandT = sb.tile([128, 8, 64], dt.bfloat16)  # [kk, (c, b, j)]
            for c in range(8):
                nc.sync.dma_start(
                    out=selcandT[:, c, :], in_=s8b[:, 128 * c:128 * (c + 1)],
                    transpose=True,
                )

            # ---------------- entity scores + context ----------------
            eb = sb.tile([128, NEC, D], dt.bfloat16)  # persistent entities bf16
            e_col = sb.tile([128, NEC], dt.float32)
            for i in range(NEC):
                nc.gpsimd.dma_start(
                    out=eb[:, i, :], in_=ents[128 * i:128 * (i + 1), :])
                prod = ring.tile([128, D], dt.bfloat16, tag="prod")
                nc.vector.tensor_tensor(out=prod, in0=eb[:, i, :], in1=wrep_e,
                                        op=OP.mult)
                junk = ring.tile([128, D], dt.bfloat16, tag="junk")
                nc.scalar.activation(out=junk, in_=prod, func=AF.Copy,
                                     accum_out=e_col[:, i:i + 1])

            exp_e = sb.tile([128, NEC], dt.bfloat16)
            ze_col = sb.tile([128, 1], dt.float32)
            nc.scalar.activation(out=exp_e, in_=e_col, func=AF.Exp,
                                 accum_out=ze_col)
            ctx_lo = psA.tile([1, 512], dt.float32, tag="acc")
            ctx_hi = psA.tile([1, 512], dt.float32, tag="acc")
            for i in range(NEC):
                nc.tensor.matmul(out=ctx_lo, lhsT=exp_e[:, i:i + 1],
                                 rhs=eb[:, i, 0:512].opt(),
                                 start=(i == 0), stop=(i == NEC - 1))
                nc.tensor.matmul(out=ctx_hi, lhsT=exp_e[:, i:i + 1],
                                 rhs=eb[:, i, 512:1024].opt(),
                                 start=(i == 0), stop=(i == NEC - 1))
            ze_ps = psC.tile([1, 1], dt.float32, tag="sm")
            nc.tensor.matmul(out=ze_ps, lhsT=ones128, rhs=ze_col,
                             start=True, stop=True)
            rz = sb.tile([1, 1], dt.float32)
            nc.vector.reciprocal(out=rz, in_=ze_ps)
            ent_ctx = sb.tile([1, D], dt.float32)
            nc.vector.tensor_scalar(out=ent_ctx[:, 0:512], in0=ctx_lo,
                                    scalar1=rz, scalar2=None, op0=OP.mult)
            nc.vector.tensor_scalar(out=ent_ctx[:, 512:1024], in0=ctx_hi,
                                    scalar1=rz, scalar2=None, op0=OP.mult)
            # q_e = ent_ctx . wp_e
            prode = sb.tile([1, D], dt.float32)
            nc.vector.tensor_tensor(out=prode, in0=ent_ctx, in1=wv[:, 3, :],
                                    op=OP.mult)
            q_e = sb.tile([1, 1], dt.float32)
            nc.vector.tensor_reduce(out=q_e, in_=prode,
                                    axis=mybir.AxisListType.X, op=OP.add)

            # AG1: ent_ctx + q_e
            pay1 = sb.tile([1, 1026], dt.float32)
            nc.vector.tensor_copy(pay1[:, 0:1024], ent_ctx)
            nc.vector.tensor_copy(pay1[:, 1024:1025], q_e)
            nc.vector.memset(pay1[:, 1025:1026], 0.0)
            ag1_in = dram.tile([1, 1026], dt.float32)
            ag1_out = dram.tile([8, 1026], dt.float32)
            nc.sync.dma_start(out=ag1_in, in_=pay1)
            nc.gpsimd.collective_compute(
                "AllGather", OP.bypass, ins=[ag1_in.opt()], outs=[ag1_out.opt()],
                replica_groups=[list(range(N_CORES))],
            )
            ctx8 = sb.tile([8, 1024], dt.float32)
            nc.sync.dma_start(
                out=ctx8,
                in_=bass.AP(tensor=ag1_out.tensor, offset=ag1_out.offset,
                            ap=[[1026, 8], [1, 1024]]))
            entT_ps = psC.tile([128, 64], dt.float32, tag="sm")
            for k in range(8):
                nc.tensor.transpose(entT_ps[:, 8 * k:8 * k + 8],
                                    ctx8[:, 128 * k:128 * (k + 1)],
                                    ident_f[:8, :8])
            xt_ent = sb.tile([128, 64], dt.bfloat16)
            nc.scalar.copy(out=xt_ent, in_=entT_ps)

            # ---------------- sentence scores ----------------
            s_col = sb.tile([128, NSC], dt.float32)
            t_col = sb.tile([128, NSC], dt.float32)
            for i in range(NSC):
                xb = ring.tile([128, D], dt.bfloat16, tag="xb")
                nc.gpsimd.dma_start(
                    out=xb, in_=sents[128 * i:128 * (i + 1), :])
                prod = ring.tile([128, D], dt.bfloat16, tag="prod")
                nc.vector.tensor_tensor(out=prod, in0=xb, in1=wrep_s, op=OP.mult)
                junk = ring.tile([128, D], dt.bfloat16, tag="junk")
                nc.scalar.activation(out=junk, in_=prod, func=AF.Copy,
                                     accum_out=s_col[:, i:i + 1])
                prod2 = ring.tile([128, D], dt.bfloat16, tag="prod")
                nc.vector.tensor_tensor(out=prod2, in0=xb, in1=wrep_t, op=OP.mult)
                junk = ring.tile([128, D], dt.bfloat16, tag="junk")
                nc.scalar.activation(out=junk, in_=prod2, func=AF.Copy,
                                     accum_out=t_col[:, i:i + 1])

            exp_s = sb.tile([128, NSC], dt.float32)
            zs_col = sb.tile([128, 2], dt.float32)
            nc.scalar.activation(out=exp_s, in_=s_col, func=AF.Exp,
                                 accum_out=zs_col[:, 1:2])
            st = sb.tile([128, NSC], dt.float32)
            nc.vector.tensor_tensor(out=st, in0=exp_s, in1=t_col, op=OP.mult)
            nc.vector.tensor_reduce(out=zs_col[:, 0:1], in_=st,
                                    axis=mybir.AxisListType.X, op=OP.add)
            nz_ps = psC.tile([1, 2], dt.float32, tag="sm")
            nc.tensor.matmul(out=nz_ps, lhsT=ones128, rhs=zs_col,
                             start=True, stop=True)
            rzs = sb.tile([1, 1], dt.float32)
            nc.vector.reciprocal(out=rzs, in_=nz_ps[:, 1:2])
            q_s = sb.tile([1, 2], dt.float32)
            nc.vector.tensor_scalar(out=q_s, in0=nz_ps, scalar1=rzs,
                                    scalar2=None, op0=OP.mult)
            ag2_in = dram.tile([1, 2], dt.float32)
            ag2_out = dram.tile([8, 2], dt.float32)
            nc.sync.dma_start(out=ag2_in, in_=q_s)
            nc.gpsimd.collective_compute(
                "AllGather", OP.bypass, ins=[ag2_in.opt()], outs=[ag2_out.opt()],
                replica_groups=[list(range(N_CORES))],
            )

            # q64[8b + j] = q[j];  q_row[0, j] = q[j]
            qe64 = sb.tile([64, 1], dt.float32)
            nc.sync.dma_start(
                out=qe64,
                in_=bass.AP(tensor=ag1_out.tensor, offset=ag1_out.offset + 1024,
                            ap=[[0, 8], [1026, 8]]))
            qs64 = sb.tile([64, 1], dt.float32)
            nc.sync.dma_start(
                out=qs64,
                in_=bass.AP(tensor=ag2_out.tensor, offset=ag2_out.offset,
                            ap=[[0, 8], [2, 8]]))
            q64 = sb.tile([64, 1], dt.float32)
            nc.vector.tensor_tensor(out=q64, in0=qe64, in1=qs64, op=OP.add)
            qe_row = sb.tile([1, 8], dt.float32)
            nc.sync.dma_start(
                out=qe_row,
                in_=bass.AP(tensor=ag1_out.tensor, offset=ag1_out.offset + 1024,
                            ap=[[0, 1], [1026, 8]]))
            qs_row = sb.tile([1, 8], dt.float32)
            nc.sync.dma_start(
                out=qs_row,
                in_=bass.AP(tensor=ag2_out.tensor, offset=ag2_out.offset,
                            ap=[[0, 1], [2, 8]]))
            q_row = sb.tile([1, 8], dt.float32)
            nc.vector.tensor_tensor(out=q_row, in0=qe_row, in1=qs_row, op=OP.add)

            # ---------------- LSTM weight slice, transposed ----------------
            WT = sb.tile([128, KCH, 512], dt.bfloat16)
            for g in range(4):
                wnat = ring.tile([128, KPAD], dt.bfloat16, tag="wnat")
                nc.gpsimd.dma_start(out=wnat,
                                    in_=wsl[128 * g:128 * (g + 1), :])
                for kc in range(KCH):
                    wt_ps = psB.tile([128, 128], dt.bfloat16, tag="med")
                    # transpose [128 g', 128 k] -> [128 k, 128 g']
                    nc.tensor.transpose(
                        wt_ps, wnat[:, 128 * kc:128 * (kc + 1)], ident_b)
                    nc.scalar.copy(out=WT[:, kc, 128 * g:128 * (g + 1)],
                                   in_=wt_ps)

            # ---------------- recurrence ----------------
            def argmax_select(z64, z_row, t, hT_bf):
                """Select winner, write output row t, build selT. Returns selT."""
                zmax64 = sb.tile([64, 1], dt.float32, tag=f"zmax{t}")
                nc.gpsimd.partition_all_reduce(
                    out_ap=zmax64, in_ap=z64, channels=64, reduce_op=ReduceOp.max)
                oh64 = sb.tile([64, 1], dt.float32, tag=f"oh64{t}")
                nc.vector.tensor_tensor(out=oh64, in0=z64, in1=zmax64,
                                        op=OP.is_equal)
                mask64 = sb.tile([64, 8], dt.float32, tag=f"mask{t}")
                nc.vector.tensor_scalar(out=mask64, in0=bmask, scalar1=oh64,
                                        scalar2=None, op0=OP.mult)
                sel_lo = psB.tile([8, 512], dt.float32, tag="med")
                sel_hi = psB.tile([8, 512], dt.float32, tag="med")
                nc.tensor.matmul(out=sel_lo, lhsT=mask64, rhs=s8f[:, 0:512],
                                 start=True, stop=True)
                nc.tensor.matmul(out=sel_hi, lhsT=mask64, rhs=s8f[:, 512:1024],
                                 start=True, stop=True)
                sel_sb = sb.tile([8, 1024], dt.float32, tag=f"sel{t}")
                nc.scalar.copy(out=sel_sb[:, 0:512], in_=sel_lo)
                nc.scalar.copy(out=sel_sb[:, 512:1024], in_=sel_hi)
                nc.sync.dma_start(out=out[:, t, :], in_=sel_sb)

                if t == 2:
                    return None
                # onehot row -> bf16 -> broadcast; selT via masked j-reduce
                zmax_r = sb.tile([1, 1], dt.float32, tag=f"zmr{t}")
                nc.vector.tensor_reduce(out=zmax_r, in_=z_row,
                                        axis=mybir.AxisListType.X, op=OP.max)
                oh_row = sb.tile([1, 8], dt.float32, tag=f"ohr{t}")
                nc.vector.tensor_tensor(out=oh_row, in0=z_row,
                                        in1=zmax_r.to_broadcast([1, 8]),
                                        op=OP.is_equal)
                ohb = sb.tile([1, 8], dt.bfloat16, tag=f"ohb{t}")
                nc.scalar.copy(out=ohb, in_=oh_row)
                oh128 = sb.tile([128, 8], dt.bfloat16, tag=f"oh128{t}")
                nc.gpsimd.partition_broadcast(out_ap=oh128, in_ap=ohb,
                                              channels=128)
                scr = sb.tile([128, 512], dt.bfloat16, tag="selscr")
                in0 = bass.AP(tensor=selcandT.tensor, offset=selcandT.offset,
                              ap=[selcandT.ap[0], [64, 8], [8, 8], [1, 8]])
                in1 = bass.AP(tensor=oh128.tensor, offset=oh128.offset,
                              ap=[oh128.ap[0], [0, 8], [0, 8], [1, 8]])
                nc.vector.tensor_tensor(out=scr, in0=in0, in1=in1, op=OP.mult)
                selT_f = sb.tile([128, 64], dt.float32, tag="selTf")
                nc.vector.tensor_reduce(
                    out=selT_f,
                    in_=bass.AP(tensor=scr.tensor, offset=scr.offset,
                                ap=[scr.ap[0], [8, 64], [1, 8]]),
                    axis=mybir.AxisListType.X, op=OP.add)
                selT = sb.tile([128, 64], dt.bfloat16, tag="selT")
                nc.scalar.copy(out=selT, in_=selT_f)
                return selT

            def lstm_step(selT, hT_bf, c_prev, step):
                """One LSTM step; returns (c_new, h_ag_out dram tile)."""
                gates = psA.tile([8, 512], dt.float32, tag="acc")
                chunks = []
                for kc in range(8):
                    chunks.append((kc, selT[:, 8 * kc:8 * (kc + 1)]))
                for kc in range(8):
                    chunks.append((8 + kc, xt_ent[:, 8 * kc:8 * (kc + 1)]))
                if hT_bf is not None:
                    for kc in range(8):
                        chunks.append((16 + kc, hT_bf[:, 8 * kc:8 * (kc + 1)]))
                chunks.append((24, bias_chunk))
                for n, (kc, lhsT) in enumerate(chunks):
                    nc.tensor.matmul(out=gates, lhsT=lhsT, rhs=WT[:, kc, :].opt(),
                                     start=(n == 0), stop=(n == len(chunks) - 1))
                sig_i = sb.tile([8, 128], dt.float32, tag="nl_i")
                nc.scalar.activation(out=sig_i, in_=gates[:, 0:128], func=AF.Sigmoid)
                tanh_g = sb.tile([8, 128], dt.float32, tag="nl_g")
                nc.scalar.activation(out=tanh_g, in_=gates[:, 256:384], func=AF.Tanh)
                sig_o = sb.tile([8, 128], dt.float32, tag="nl_o")
                nc.scalar.activation(out=sig_o, in_=gates[:, 384:512], func=AF.Sigmoid)
                ig = sb.tile([8, 128], dt.float32, tag=f"cst{step}")
                nc.vector.tensor_tensor(out=ig, in0=sig_i, in1=tanh_g, op=OP.mult)
                if c_prev is None:
                    c_new = ig
                else:
                    sig_f = sb.tile([8, 128], dt.float32, tag="nl_f")
                    nc.scalar.activation(out=sig_f, in_=gates[:, 128:256],
                                         func=AF.Sigmoid)
                    fc = sb.tile([8, 128], dt.float32, tag="nl_fc")
                    nc.vector.tensor_tensor(out=fc, in0=sig_f, in1=c_prev,
                                            op=OP.mult)
                    c_new = sb.tile([8, 128], dt.float32, tag=f"cst{step}b")
                    nc.vector.tensor_tensor(out=c_new, in0=fc, in1=ig, op=OP.add)
                tanh_c = sb.tile([8, 128], dt.float32, tag="nl_tc")
                nc.scalar.activation(out=tanh_c, in_=c_new, func=AF.Tanh)
                h_sb = sb.tile([8, 128], dt.float32, tag=f"hst{step}")
                nc.vector.tensor_tensor(out=h_sb, in0=sig_o, in1=tanh_c, op=OP.mult)

                hprod = sb.tile([8, 128], dt.float32, tag="nl_hpr")
                nc.vector.tensor_tensor(out=hprod, in0=h_sb, in1=wph8, op=OP.mult)
                hp_own = sb.tile([8, 1], dt.float32, tag=f"hp{step}")
                nc.vector.tensor_reduce(out=hp_own, in_=hprod,
                                        axis=mybir.AxisListType.X, op=OP.add)
                hT_ps = psC.tile([128, 8], dt.float32, tag="sm")
                nc.tensor.transpose(hT_ps, h_sb, ident_f[:8, :8])
                hpay = sb.tile([128, 9], dt.float32, tag=f"hpay{step}")
                nc.scalar.copy(out=hpay[:, 0:8], in_=hT_ps)
                nc.vector.tensor_copy(hpay[0:8, 8:9], hp_own)
                h_in = dram.tile([128, 9], dt.float32, tag=f"hin{step}")
                h_out = dram.tile([1024, 9], dt.float32, tag=f"hout{step}")
                nc.sync.dma_start(out=h_in, in_=hpay)
                nc.gpsimd.collective_compute(
                    "AllGather", OP.bypass, ins=[h_in.opt()], outs=[h_out.opt()],
                    replica_groups=[list(range(N_CORES))],
                )
                return c_new, h_out

            def read_h_ag(h_out, step):
                hT8 = sb.tile([128, 64], dt.float32, tag=f"hT8{step}")
                nc.sync.dma_start(
                    out=hT8,
                    in_=bass.AP(tensor=h_out.tensor, offset=h_out.offset,
                                ap=[[9, 128], [9 * 128, 8], [1, 8]]))
                hT_bf = sb.tile([128, 64], dt.bfloat16, tag=f"hTb{step}")
                nc.scalar.copy(out=hT_bf, in_=hT8)
                hp_cb = sb.tile([8, 8], dt.float32, tag=f"hpcb{step}")
                nc.sync.dma_start(
                    out=hp_cb,
                    in_=bass.AP(tensor=h_out.tensor, offset=h_out.offset + 8,
                                ap=[[9 * 128, 8], [9, 8]]))
                return hT_bf, hp_cb

            def z_from_hp(hp_cb, step):
                z64_ps = psC.tile([64, 1], dt.float32, tag="sm")
                hp64 = sb.tile([8, 64], dt.float32, tag=f"hp64{step}")
                nc.vector.tensor_copy(
                    hp64,
                    bass.AP(tensor=hp_cb.tensor, offset=hp_cb.offset,
                            ap=[hp_cb.ap[0], [0, 8], [1, 8]]))
                nc.tensor.matmul(out=z64_ps, lhsT=hp64, rhs=ones128[0:8, :],
                                 start=True, stop=True)
                z64 = sb.tile([64, 1], dt.float32, tag=f"z64{step}")
                nc.vector.tensor_tensor(out=z64, in0=z64_ps, in1=q64, op=OP.add)
                zr_ps = psC.tile([1, 8], dt.float32, tag="sm")
                nc.tensor.matmul(out=zr_ps, lhsT=ones128[0:8, :], rhs=hp_cb,
                                 start=True, stop=True)
                z_row = sb.tile([1, 8], dt.float32, tag=f"zrow{step}")
                nc.vector.tensor_tensor(out=z_row, in0=zr_ps, in1=q_row, op=OP.add)
                return z64, z_row

            # step 0: z = q
            selT0 = argmax_select(q64, q_row, 0, None)
            c1, h_out1 = lstm_step(selT0, None, None, 1)
            hT_bf1, hp_cb1 = read_h_ag(h_out1, 1)
            z64_1, zrow_1 = z_from_hp(hp_cb1, 1)
            selT1 = argmax_select(z64_1, zrow_1, 1, hT_bf1)
            c2, h_out2 = lstm_step(selT1, hT_bf1, c1, 2)
            hT_bf2, hp_cb2 = read_h_ag(h_out2, 2)
            z64_2, zrow_2 = z_from_hp(hp_cb2, 2)
            argmax_select(z64_2, zrow_2, 2, None)

    nc.compile()
    return nc


def _prep_inputs(inputs):
    sents = np.asarray(inputs["sents"], np.float32)
    ents = np.asarray(inputs["entities"], np.float32)
    Wae = np.asarray(inputs["Wae"], np.float32)
    Was = np.asarray(inputs["Was"], np.float32)
    Wp = np.asarray(inputs["Wp"], np.float32)
    W_ih = np.asarray(inputs["W_ih"], np.float32)
    W_hh = np.asarray(inputs["W_hh"], np.float32)
    b_ih = np.asarray(inputs["b_ih"], np.float32)
    b_hh = np.asarray(inputs["b_hh"], np.float32)

    s8 = np.ascontiguousarray(sents[:, 0:8, :].reshape(64, D))
    wfull = np.zeros((4 * D, KPAD), np.float32)
    wfull[:, 0:2 * D] = W_ih
    wfull[:, 2 * D:3 * D] = W_hh
    wfull[:, 3 * D] = b_ih + b_hh

    in_maps = []
    for c in range(N_CORES):
        rows = np.concatenate(
            [np.arange(D * t + 128 * c, D * t + 128 * (c + 1)) for t in range(4)])
        wslice = np.ascontiguousarray(wfull[rows])
        wv = np.zeros((1, 8, D), np.float32)
        wv[0, 0] = Was[D:2 * D, 0]
        wv[0, 1] = Wp[2 * D:3 * D, 0]
        wv[0, 2] = Wae[D:2 * D, 0]
        wv[0, 3] = Wp[D:2 * D, 0]
        wv[0, 4, 0:128] = Wp[128 * c:128 * (c + 1), 0]
        in_maps.append({
            "sents": np.ascontiguousarray(sents[c]),
            "ents": np.ascontiguousarray(ents[c]),
            "s8": s8,
            "wsl": wslice,
            "wvecs": wv,
        })
    return in_maps


def get_compiled():
    if "nc" not in _CACHE:
        _CACHE["nc"] = _build()
    return _CACHE["nc"]


def kernel(**inputs) -> np.ndarray:
    from concourse import bass_utils

    nc = get_compiled()
    in_maps = _prep_inputs(inputs)
    res = bass_utils.run_bass_kernel_spmd(
        nc, in_maps, core_ids=list(range(N_CORES)))
    return res.results[0]["out"]


# revision 10
# speedup vs baseline: 1.1284x; 1.1284x over previous
"""Trainium2 Bass kernel for nn_ContentSelector (topk_masking).

Math refactoring (validated against the reference in proto.py):
  - The attention query term h @ W[:dq] adds a per-batch constant to every
    score, and softmax is shift-invariant => attention weights are
    independent of h. ent_ctx / sent_ctx are therefore step-invariant and
    computed once. Biases bae/bas shift scores uniformly (dropped); bp and
    the sigmoid are monotonic (argmax-invariant, dropped).
  - sent_ctx is only consumed through q_s = sent_ctx . wp_s
      q_s = sum_n softmax(s)_n * t_n,  s_n = sents[n].was_m, t_n = sents[n].wp_s
    so sents needs only two matvecs (no [B,D] weighted sum).
  - top_idx = argmax_b p_b selects one of the first 8 sentences; output rows
    are exact f32 copies of input rows.

Distribution: data-parallel over batch for the attention phase (core c owns
batch c); the LSTM weight matrix is output-dim sharded (core c owns 128 of
each gate's 1024 dims) with one small AllGather of (h-slice^T, partial
h.wp_h) per recurrence step. Scores z = h.wp_h + q are computed redundantly
on every core so the argmax needs no extra communication.

On-chip dataflow: f32 HBM reads are cast to bf16 during DMA; score matvecs
run as DVE tensor_tensor multiplies + ACT copy-with-accum row sums (the
fused tensor_tensor_reduce custom-DVE op crashes this runtime); weighted
sums / gates / transposes run on the tensor engine with f32 PSUM accum.
"""
import numpy as np

B = 8
NS = 4096
NE = 1024
D = 1024
N_CORES = 8
KCH = 25          # 24 contraction chunks of 128 + 1 bias chunk
KPAD = KCH * 128  # 3200

_CACHE = {}


def _build():
    import concourse.bacc as bacc
    import concourse.bass as bass
    import concourse.mybir as mybir
    import concourse.tile as tile
    from concourse.bass_isa import ReduceOp

    dt = mybir.dt
    AF = mybir.ActivationFunctionType
    OP = mybir.AluOpType

    nc = bacc.Bacc(
        "TRN2", target_bir_lowering=False, debug=False,
        enable_asserts=True, num_devices=N_CORES,
    )

    sents = nc.dram_tensor("sents", [NS, D], dt.float32, kind="ExternalInput").ap()
    ents = nc.dram_tensor("ents", [NE, D], dt.float32, kind="ExternalInput").ap()
    s8 = nc.dram_tensor("s8", [64, D], dt.float32, kind="ExternalInput").ap()
    wsl = nc.dram_tensor("wsl", [512, KPAD], dt.float32, kind="ExternalInput").ap()
    wvecs = nc.dram_tensor("wvecs", [1, 5, D], dt.float32, kind="ExternalInput").ap()
    out = nc.dram_tensor("out", [B, 3, D], dt.float32, kind="ExternalOutput").ap()

    NSC = NS // 128  # 32 sent chunks
    NEC = NE // 128  # 8 entity chunks

    with tile.TileContext(nc) as tc:
        with (
            tc.tile_pool(name="sb", bufs=1) as sb,
            tc.tile_pool(name="ring", bufs=4) as ring,
            tc.tile_pool(name="ring2", bufs=2) as ring2,
            tc.tile_pool(name="psA", bufs=2, space="PSUM") as psA,
            tc.tile_pool(name="psB", bufs=2, space="PSUM") as psB,
            tc.tile_pool(name="psC", bufs=2, space="PSUM") as psC,
            tc.tile_pool(name="dram", bufs=1, space="DRAM") as dram,
        ):
            # ---------------- constants ----------------
            ident_f = sb.tile([128, 128], dt.float32)
            onesq = sb.tile([128, 128], dt.float32)
            nc.vector.memset(onesq, 1.0)
            nc.gpsimd.affine_select(
                out=ident_f, in_=onesq, pattern=[[-1, 128]],
                compare_op=OP.is_equal, fill=0.0, base=0, channel_multiplier=1,
            )
            ident_b = sb.tile([128, 128], dt.bfloat16)
            nc.scalar.copy(out=ident_b, in_=ident_f)

            ones128 = sb.tile([128, 1], dt.float32)
            nc.vector.memset(ones128, 1.0)

            # bmask[p, m] = 1 iff 0 <= p - 8m < 8   (only is_ge is implemented)
            bmask = sb.tile([64, 8], dt.float32)
            bm_a = sb.tile([64, 8], dt.float32)
            bm_b = sb.tile([64, 8], dt.float32)
            nc.gpsimd.affine_select(
                out=bm_a, in_=onesq[0:64, 0:8], pattern=[[-8, 8]],
                compare_op=OP.is_ge, fill=0.0, base=0, channel_multiplier=1,
            )
            nc.gpsimd.affine_select(
                out=bm_b, in_=onesq[0:64, 0:8], pattern=[[8, 8]],
                compare_op=OP.is_ge, fill=0.0, base=7, channel_multiplier=-1,
            )
            nc.vector.tensor_tensor(out=bmask, in0=bm_a, in1=bm_b, op=OP.mult)

            bias_chunk = sb.tile([128, 8], dt.bfloat16)
            nc.vector.memset(bias_chunk, 0.0)
            nc.vector.memset(bias_chunk[0:1, :], 1.0)

            # small weight vectors, replicated across partitions in bf16
            wv = sb.tile([1, 5, D], dt.float32)
            nc.sync.dma_start(out=wv, in_=wvecs)

            def rep_bf(row):
                r = sb.tile([1, D], dt.bfloat16, tag=f"repb{row}")
                nc.scalar.copy(out=r, in_=wv[:, row, :])
                full = sb.tile([128, D], dt.bfloat16, tag=f"repf{row}")
                nc.gpsimd.partition_broadcast(out_ap=full, in_ap=r, channels=128)
                return full

            wrep_s = rep_bf(0)   # was_m
            wrep_e = rep_bf(2)   # wae_m
            # wp_h slice for this core, on 8 partitions (f32)
            wph_row = sb.tile([1, 128], dt.float32)
            nc.vector.tensor_copy(wph_row, wv[:, 4, 0:128])
            wph8 = sb.tile([8, 128], dt.float32)
            nc.gpsimd.partition_broadcast(out_ap=wph8, in_ap=wph_row, channels=8)

            # ---------------- candidate sentences ----------------
            s8f = sb.tile([64, D], dt.float32)
            nc.sync.dma_start(out=s8f, in_=s8)
            s8b = sb.tile([64, D], dt.bfloat16)
            nc.scalar.copy(out=s8b, in_=s8f)
            selcandT = sb.tile([128, 8, 64], dt.bfloat16)  # [kk, (c, b, j)]
            for c in range(8):
                nc.sync.dma_start(
                    out=selcandT[:, c, :], in_=s8b[:, 128 * c:128 * (c + 1)],
                    transpose=True,
                )

            # ---------------- entity scores + context ----------------
            eb = sb.tile([128, NEC, D], dt.bfloat16)  # persistent entities bf16
            e_col = sb.tile([128, NEC], dt.float32)
            junk = sb.tile([128, D], dt.bfloat16)
            for i in range(NEC):
                nc.gpsimd.dma_start(
                    out=eb[:, i, :], in_=ents[128 * i:128 * (i + 1), :])
                prod = ring.tile([128, D], dt.bfloat16, tag="prod")
                nc.vector.tensor_tensor(out=prod, in0=eb[:, i, :].opt(),
                                        in1=wrep_e, op=OP.mult)
                if i % 2 == 0:
                    nc.scalar.activation(out=junk, in_=prod, func=AF.Copy,
                                         accum_out=e_col[:, i:i + 1])
                else:
                    nc.vector.tensor_reduce(out=e_col[:, i:i + 1], in_=prod,
                                            axis=mybir.AxisListType.X, op=OP.add)

            exp_e = sb.tile([128, NEC], dt.bfloat16)
            ze_col = sb.tile([128, 1], dt.float32)
            nc.scalar.activation(out=exp_e, in_=e_col, func=AF.Exp,
                                 accum_out=ze_col)
            ctx_lo = psA.tile([1, 512], dt.float32, tag="acc")
            ctx_hi = psA.tile([1, 512], dt.float32, tag="acc")
            for i in range(NEC):
                nc.tensor.matmul(out=ctx_lo, lhsT=exp_e[:, i:i + 1],
                                 rhs=eb[:, i, 0:512].opt(),
                                 start=(i == 0), stop=(i == NEC - 1))
                nc.tensor.matmul(out=ctx_hi, lhsT=exp_e[:, i:i + 1],
                                 rhs=eb[:, i, 512:1024].opt(),
                                 start=(i == 0), stop=(i == NEC - 1))
            ze_ps = psC.tile([1, 1], dt.float32, tag="sm")
            nc.tensor.matmul(out=ze_ps, lhsT=ones128, rhs=ze_col,
                             start=True, stop=True)
            rz = sb.tile([1, 1], dt.float32)
            nc.vector.reciprocal(out=rz, in_=ze_ps)
            ent_ctx = sb.tile([1, D], dt.float32)
            nc.vector.tensor_scalar(out=ent_ctx[:, 0:512], in0=ctx_lo,
                                    scalar1=rz, scalar2=None, op0=OP.mult)
            nc.vector.tensor_scalar(out=ent_ctx[:, 512:1024], in0=ctx_hi,
                                    scalar1=rz, scalar2=None, op0=OP.mult)
            # q_e = ent_ctx . wp_e
            prode = sb.tile([1, D], dt.float32)
            nc.vector.tensor_tensor(out=prode, in0=ent_ctx, in1=wv[:, 3, :],
                                    op=OP.mult)
            q_e = sb.tile([1, 1], dt.float32)
            nc.vector.tensor_reduce(out=q_e, in_=prode,
                                    axis=mybir.AxisListType.X, op=OP.add)

            # AG1: ent_ctx + q_e
            pay1 = sb.tile([1, 1026], dt.float32)
            nc.vector.tensor_copy(pay1[:, 0:1024], ent_ctx)
            nc.vector.tensor_copy(pay1[:, 1024:1025], q_e)
            nc.vector.memset(pay1[:, 1025:1026], 0.0)
            ag1_in = dram.tile([1, 1026], dt.float32)
            ag1_out = dram.tile([8, 1026], dt.float32)
            nc.sync.dma_start(out=ag1_in, in_=pay1)
            nc.gpsimd.collective_compute(
                "AllGather", OP.bypass, ins=[ag1_in.opt()], outs=[ag1_out.opt()],
                replica_groups=[list(range(N_CORES))],
            )
            ctx8 = sb.tile([8, 1024], dt.float32)
            nc.sync.dma_start(
                out=ctx8,
                in_=bass.AP(tensor=ag1_out.tensor, offset=ag1_out.offset,
                            ap=[[1026, 8], [1, 1024]]))
            entT_ps = psC.tile([128, 64], dt.float32, tag="sm")
            for k in range(8):
                nc.tensor.transpose(entT_ps[:, 8 * k:8 * k + 8],
                                    ctx8[:, 128 * k:128 * (k + 1)],
                                    ident_f[:8, :8])
            xt_ent = sb.tile([128, 64], dt.bfloat16)
            nc.scalar.copy(out=xt_ent, in_=entT_ps)

            # ---------------- sentence scores ----------------
            # chunks stay resident in SBUF (bf16): one HBM pass feeds both the
            # score matvec and the PE weighted sum below.
            sres = sb.tile([128, NSC, D], dt.bfloat16)
            s_col = sb.tile([128, NSC], dt.float32)
            for i in range(NSC):
                nc.gpsimd.dma_start(
                    out=sres[:, i, :], in_=sents[128 * i:128 * (i + 1), :])
                prod = ring.tile([128, D], dt.bfloat16, tag="prod")
                nc.vector.tensor_tensor(out=prod, in0=sres[:, i, :].opt(),
                                        in1=wrep_s, op=OP.mult)
                if i % 2 == 0:
                    nc.scalar.activation(out=junk, in_=prod, func=AF.Copy,
                                         accum_out=s_col[:, i:i + 1])
                else:
                    nc.vector.tensor_reduce(out=s_col[:, i:i + 1], in_=prod,
                                            axis=mybir.AxisListType.X, op=OP.add)

            exp_s = sb.tile([128, NSC], dt.bfloat16)
            zs_col = sb.tile([128, 1], dt.float32)
            nc.scalar.activation(out=exp_s, in_=s_col, func=AF.Exp,
                                 accum_out=zs_col)
            # unnormalized sent context V = exp_s^T @ sents  (PE, contract n)
            sctx_lo = psA.tile([1, 512], dt.float32, tag="acc")
            sctx_hi = psA.tile([1, 512], dt.float32, tag="acc")
            for i in range(NSC):
                nc.tensor.matmul(out=sctx_lo, lhsT=exp_s[:, i:i + 1],
                                 rhs=sres[:, i, 0:512].opt(),
                                 start=(i == 0), stop=(i == NSC - 1))
                nc.tensor.matmul(out=sctx_hi, lhsT=exp_s[:, i:i + 1],
                                 rhs=sres[:, i, 512:1024].opt(),
                                 start=(i == 0), stop=(i == NSC - 1))
            # q_s = (V . wp_s) / Z  -- only the scalar is ever needed
            vdot = sb.tile([1, D], dt.float32)
            nc.vector.tensor_tensor(out=vdot[:, 0:512], in0=sctx_lo,
                                    in1=wv[:, 1, 0:512], op=OP.mult)
            nc.vector.tensor_tensor(out=vdot[:, 512:1024], in0=sctx_hi,
                                    in1=wv[:, 1, 512:1024], op=OP.mult)
            num_s = sb.tile([1, 1], dt.float32)
            nc.vector.tensor_reduce(out=num_s, in_=vdot,
                                    axis=mybir.AxisListType.X, op=OP.add)
            zzs = psC.tile([1, 1], dt.float32, tag="sm")
            nc.tensor.matmul(out=zzs, lhsT=ones128, rhs=zs_col,
                             start=True, stop=True)
            rzs = sb.tile([1, 1], dt.float32)
            nc.vector.reciprocal(out=rzs, in_=zzs)
            q_s = sb.tile([1, 2], dt.float32)
            nc.vector.tensor_tensor(out=q_s[:, 0:1], in0=num_s, in1=rzs,
                                    op=OP.mult)
            nc.vector.memset(q_s[:, 1:2], 0.0)
            ag2_in = dram.tile([1, 2], dt.float32)
            ag2_out = dram.tile([8, 2], dt.float32)
            nc.sync.dma_start(out=ag2_in, in_=q_s)
            nc.gpsimd.collective_compute(
                "AllGather", OP.bypass, ins=[ag2_in.opt()], outs=[ag2_out.opt()],
                replica_groups=[list(range(N_CORES))],
            )

            # q64[8b + j] = q[j];  q_row[0, j] = q[j]
            qe64 = sb.tile([64, 1], dt.float32)
            nc.sync.dma_start(
                out=qe64,
                in_=bass.AP(tensor=ag1_out.tensor, offset=ag1_out.offset + 1024,
                            ap=[[0, 8], [1026, 8]]))
            qs64 = sb.tile([64, 1], dt.float32)
            nc.sync.dma_start(
                out=qs64,
                in_=bass.AP(tensor=ag2_out.tensor, offset=ag2_out.offset,
                            ap=[[0, 8], [2, 8]]))
            q64 = sb.tile([64, 1], dt.float32)
            nc.vector.tensor_tensor(out=q64, in0=qe64, in1=qs64, op=OP.add)
            qe_row = sb.tile([1, 8], dt.float32)
            nc.sync.dma_start(
                out=qe_row,
                in_=bass.AP(tensor=ag1_out.tensor, offset=ag1_out.offset + 1024,
                            ap=[[0, 1], [1026, 8]]))
            qs_row = sb.tile([1, 8], dt.float32)
            nc.sync.dma_start(
                out=qs_row,
                in_=bass.AP(tensor=ag2_out.tensor, offset=ag2_out.offset,
                            ap=[[0, 1], [2, 8]]))
            q_row = sb.tile([1, 8], dt.float32)
            nc.vector.tensor_tensor(out=q_row, in0=qe_row, in1=qs_row, op=OP.add)

            # ---------------- LSTM weight slice, transposed ----------------
            WT = sb.tile([128, KCH, 512], dt.bfloat16)
            for g in range(4):
                wnat = ring2.tile([128, KPAD], dt.bfloat16, tag="wnat")
                nc.gpsimd.dma_start(out=wnat,
                                    in_=wsl[128 * g:128 * (g + 1), :])
                for kc0 in range(0, KCH, 4):
                    kn = min(4, KCH - kc0)
                    wt_ps = psB.tile([128, 512], dt.bfloat16, tag="med")
                    for kk in range(kn):
                        # transpose [128 g', 128 k] -> [128 k, 128 g']
                        nc.tensor.transpose(
                            wt_ps[:, 128 * kk:128 * (kk + 1)],
                            wnat[:, 128 * (kc0 + kk):128 * (kc0 + kk + 1)],
                            ident_b)
                    dst = bass.AP(
                        tensor=WT.tensor,
                        offset=WT[:, kc0, 128 * g:128 * (g + 1)].offset,
                        ap=[WT.ap[0], [512, kn], [1, 128]])
                    nc.scalar.copy(out=dst, in_=wt_ps[:, 0:128 * kn])

            # ---------------- recurrence ----------------
            def argmax_select(z64, z_row, t, hT_bf):
                """Select winner, write output row t, build selT. Returns selT."""
                zmax64 = sb.tile([64, 1], dt.float32, tag=f"zmax{t}")
                nc.gpsimd.partition_all_reduce(
                    out_ap=zmax64, in_ap=z64, channels=64, reduce_op=ReduceOp.max)
                oh64 = sb.tile([64, 1], dt.float32, tag=f"oh64{t}")
                nc.vector.tensor_tensor(out=oh64, in0=z64, in1=zmax64,
                                        op=OP.is_equal)
                mask64 = sb.tile([64, 8], dt.float32, tag=f"mask{t}")
                nc.vector.tensor_scalar(out=mask64, in0=bmask, scalar1=oh64,
                                        scalar2=None, op0=OP.mult)
                sel_lo = psB.tile([8, 512], dt.float32, tag="med")
                sel_hi = psB.tile([8, 512], dt.float32, tag="med")
                nc.tensor.matmul(out=sel_lo, lhsT=mask64, rhs=s8f[:, 0:512],
                                 start=True, stop=True)
                nc.tensor.matmul(out=sel_hi, lhsT=mask64, rhs=s8f[:, 512:1024],
                                 start=True, stop=True)
                sel_sb = sb.tile([8, 1024], dt.float32, tag=f"sel{t}")
                nc.scalar.copy(out=sel_sb[:, 0:512], in_=sel_lo)
                nc.scalar.copy(out=sel_sb[:, 512:1024], in_=sel_hi)
                nc.sync.dma_start(out=out[:, t, :], in_=sel_sb)

                if t == 2:
                    return None
                # onehot row -> bf16 -> broadcast; selT via masked j-reduce
                zmax_r = sb.tile([1, 1], dt.float32, tag=f"zmr{t}")
                nc.vector.tensor_reduce(out=zmax_r, in_=z_row,
                                        axis=mybir.AxisListType.X, op=OP.max)
                oh_row = sb.tile([1, 8], dt.float32, tag=f"ohr{t}")
                nc.vector.tensor_tensor(out=oh_row, in0=z_row,
                                        in1=zmax_r.to_broadcast([1, 8]),
                                        op=OP.is_equal)
                ohb = sb.tile([1, 8], dt.bfloat16, tag=f"ohb{t}")
                nc.scalar.copy(out=ohb, in_=oh_row)
                oh128 = sb.tile([128, 8], dt.bfloat16, tag=f"oh128{t}")
                nc.gpsimd.partition_broadcast(out_ap=oh128, in_ap=ohb,
                                              channels=128)
                scr = sb.tile([128, 512], dt.bfloat16, tag="selscr")
                in0 = bass.AP(tensor=selcandT.tensor, offset=selcandT.offset,
                              ap=[selcandT.ap[0], [64, 8], [8, 8], [1, 8]])
                in1 = bass.AP(tensor=oh128.tensor, offset=oh128.offset,
                              ap=[oh128.ap[0], [0, 8], [0, 8], [1, 8]])
                nc.vector.tensor_tensor(out=scr, in0=in0, in1=in1, op=OP.mult)
                selT_f = sb.tile([128, 64], dt.float32, tag="selTf")
                nc.vector.tensor_reduce(
                    out=selT_f,
                    in_=bass.AP(tensor=scr.tensor, offset=scr.offset,
                                ap=[scr.ap[0], [8, 64], [1, 8]]),
                    axis=mybir.AxisListType.X, op=OP.add)
                selT = sb.tile([128, 64], dt.bfloat16, tag="selT")
                nc.scalar.copy(out=selT, in_=selT_f)
                return selT

            def lstm_step(selT, hT_bf, c_prev, step):
                """One LSTM step; returns (c_new, h_ag_out dram tile)."""
                gates = psA.tile([8, 512], dt.float32, tag="acc")
                chunks = []
                for kc in range(8):
                    chunks.append((kc, selT[:, 8 * kc:8 * (kc + 1)]))
                for kc in range(8):
                    chunks.append((8 + kc, xt_ent[:, 8 * kc:8 * (kc + 1)]))
                if hT_bf is not None:
                    for kc in range(8):
                        chunks.append((16 + kc, hT_bf[:, 8 * kc:8 * (kc + 1)]))
                chunks.append((24, bias_chunk))
                for n, (kc, lhsT) in enumerate(chunks):
                    nc.tensor.matmul(out=gates, lhsT=lhsT, rhs=WT[:, kc, :].opt(),
                                     start=(n == 0), stop=(n == len(chunks) - 1))
                sig_i = sb.tile([8, 128], dt.float32, tag="nl_i")
                nc.scalar.activation(out=sig_i, in_=gates[:, 0:128], func=AF.Sigmoid)
                tanh_g = sb.tile([8, 128], dt.float32, tag="nl_g")
                nc.scalar.activation(out=tanh_g, in_=gates[:, 256:384], func=AF.Tanh)
                sig_o = sb.tile([8, 128], dt.float32, tag="nl_o")
                nc.scalar.activation(out=sig_o, in_=gates[:, 384:512], func=AF.Sigmoid)
                ig = sb.tile([8, 128], dt.float32, tag=f"cst{step}")
                nc.vector.tensor_tensor(out=ig, in0=sig_i, in1=tanh_g, op=OP.mult)
                if c_prev is None:
                    c_new = ig
                else:
                    sig_f = sb.tile([8, 128], dt.float32, tag="nl_f")
                    nc.scalar.activation(out=sig_f, in_=gates[:, 128:256],
                                         func=AF.Sigmoid)
                    fc = sb.tile([8, 128], dt.float32, tag="nl_fc")
                    nc.vector.tensor_tensor(out=fc, in0=sig_f, in1=c_prev,
                                            op=OP.mult)
                    c_new = sb.tile([8, 128], dt.float32, tag=f"cst{step}b")
                    nc.vector.tensor_tensor(out=c_new, in0=fc, in1=ig, op=OP.add)
                tanh_c = sb.tile([8, 128], dt.float32, tag="nl_tc")
                nc.scalar.activation(out=tanh_c, in_=c_new, func=AF.Tanh)
                h_sb = sb.tile([8, 128], dt.float32, tag=f"hst{step}")
                nc.vector.tensor_tensor(out=h_sb, in0=sig_o, in1=tanh_c, op=OP.mult)

                hprod = sb.tile([8, 128], dt.float32, tag="nl_hpr")
                nc.vector.tensor_tensor(out=hprod, in0=h_sb, in1=wph8, op=OP.mult)
                hp_own = sb.tile([8, 1], dt.float32, tag=f"hp{step}")
                nc.vector.tensor_reduce(out=hp_own, in_=hprod,
                                        axis=mybir.AxisListType.X, op=OP.add)
                hT_ps = psC.tile([128, 8], dt.float32, tag="sm")
                nc.tensor.transpose(hT_ps, h_sb, ident_f[:8, :8])
                hpay = sb.tile([128, 9], dt.float32, tag=f"hpay{step}")
                nc.scalar.copy(out=hpay[:, 0:8], in_=hT_ps)
                nc.vector.tensor_copy(hpay[0:8, 8:9], hp_own)
                h_in = dram.tile([128, 9], dt.float32, tag=f"hin{step}")
                h_out = dram.tile([1024, 9], dt.float32, tag=f"hout{step}")
                nc.sync.dma_start(out=h_in, in_=hpay)
                nc.gpsimd.collective_compute(
                    "AllGather", OP.bypass, ins=[h_in.opt()], outs=[h_out.opt()],
                    replica_groups=[list(range(N_CORES))],
                )
                return c_new, h_out

            def read_h_ag(h_out, step):
                hT8 = sb.tile([128, 64], dt.float32, tag=f"hT8{step}")
                nc.sync.dma_start(
                    out=hT8,
                    in_=bass.AP(tensor=h_out.tensor, offset=h_out.offset,
                                ap=[[9, 128], [9 * 128, 8], [1, 8]]))
                hT_bf = sb.tile([128, 64], dt.bfloat16, tag=f"hTb{step}")
                nc.scalar.copy(out=hT_bf, in_=hT8)
                hp_cb = sb.tile([8, 8], dt.float32, tag=f"hpcb{step}")
                nc.sync.dma_start(
                    out=hp_cb,
                    in_=bass.AP(tensor=h_out.tensor, offset=h_out.offset + 8,
                                ap=[[9 * 128, 8], [9, 8]]))
                return hT_bf, hp_cb

            def z_from_hp(hp_cb, step):
                z64_ps = psC.tile([64, 1], dt.float32, tag="sm")
                hp64 = sb.tile([8, 64], dt.float32, tag=f"hp64{step}")
                nc.vector.tensor_copy(
                    hp64,
                    bass.AP(tensor=hp_cb.tensor, offset=hp_cb.offset,
                            ap=[hp_cb.ap[0], [0, 8], [1, 8]]))
                nc.tensor.matmul(out=z64_ps, lhsT=hp64, rhs=ones128[0:8, :],
                                 start=True, stop=True)
                z64 = sb.tile([64, 1], dt.float32, tag=f"z64{step}")
                nc.vector.tensor_tensor(out=z64, in0=z64_ps, in1=q64, op=OP.add)
                zr_ps = psC.tile([1, 8], dt.float32, tag="sm")
                nc.tensor.matmul(out=zr_ps, lhsT=ones128[0:8, :], rhs=hp_cb,
                                 start=True, stop=True)
                z_row = sb.tile([1, 8], dt.float32, tag=f"zrow{step}")
                nc.vector.tensor_tensor(out=z_row, in0=zr_ps, in1=q_row, op=OP.add)
                return z64, z_row

            # step 0: z = q
            selT0 = argmax_select(q64, q_row, 0, None)
            c1, h_out1 = lstm_step(selT0, None, None, 1)
            hT_bf1, hp_cb1 = read_h_ag(h_out1, 1)
            z64_1, zrow_1 = z_from_hp(hp_cb1, 1)
            selT1 = argmax_select(z64_1, zrow_1, 1, hT_bf1)
            c2, h_out2 = lstm_step(selT1, hT_bf1, c1, 2)
            hT_bf2, hp_cb2 = read_h_ag(h_out2, 2)
            z64_2, zrow_2 = z_from_hp(hp_cb2, 2)
            argmax_select(z64_2, zrow_2, 2, None)

    nc.compile()
    return nc


def _prep_inputs(inputs):
    sents = np.asarray(inputs["sents"], np.float32)
    ents = np.asarray(inputs["entities"], np.float32)
    Wae = np.asarray(inputs["Wae"], np.float32)
    Was = np.asarray(inputs["Was"], np.float32)
    Wp = np.asarray(inputs["Wp"], np.float32)
    W_ih = np.asarray(inputs["W_ih"], np.float32)
    W_hh = np.asarray(inputs["W_hh"], np.float32)
    b_ih = np.asarray(inputs["b_ih"], np.float32)
    b_hh = np.asarray(inputs["b_hh"], np.float32)

    s8 = np.ascontiguousarray(sents[:, 0:8, :].reshape(64, D))
    wfull = np.zeros((4 * D, KPAD), np.float32)
    wfull[:, 0:2 * D] = W_ih
    wfull[:, 2 * D:3 * D] = W_hh
    wfull[:, 3 * D] = b_ih + b_hh

    in_maps = []
    for c in range(N_CORES):
        rows = np.concatenate(
            [np.arange(D * t + 128 * c, D * t + 128 * (c + 1)) for t in range(4)])
        wslice = np.ascontiguousarray(wfull[rows])
        wv = np.zeros((1, 5, D), np.float32)
        wv[0, 0] = Was[D:2 * D, 0]
        wv[0, 1] = Wp[2 * D:3 * D, 0]
        wv[0, 2] = Wae[D:2 * D, 0]
        wv[0, 3] = Wp[D:2 * D, 0]
        wv[0, 4, 0:128] = Wp[128 * c:128 * (c + 1), 0]
        in_maps.append({
            "sents": np.ascontiguousarray(sents[c]),
            "ents": np.ascontiguousarray(ents[c]),
            "s8": s8,
            "wsl": wslice,
            "wvecs": wv,
        })
    return in_maps


def get_compiled():
    if "nc" not in _CACHE:
        _CACHE["nc"] = _build()
    return _CACHE["nc"]


def kernel(**inputs) -> np.ndarray:
    from concourse import bass_utils

    nc = get_compiled()
    in_maps = _prep_inputs(inputs)
    res = bass_utils.run_bass_kernel_spmd(
        nc, in_maps, core_ids=list(range(N_CORES)))
    return res.results[0]["out"]
